# revision 28
# baseline (speedup 1.0000x reference)
"""DiagonalBandAttention Trainium2 kernel.

Computation (reference semantics):
  band[b,c,j]  = mean_{k=0..20} xpad[b,c,j+k,j]        (rows zero-padded by 10)
  conv[b,c,s]  = depthwise_conv1d(band, conv_w, k=7, pad=3)   (cross-correlation)
  attn[b,d,s]  = softmax_s( sum_c point_w[d,c]*conv[b,c,s] + point_b[d] )
  out          = x, with out[b,c,j,j] = x[b,c,j,j] * attn[b,c,j]

Output is x copied verbatim except the main diagonal of each [S,S] map.
Sharding (8 cores): core k handles batch b = k//4, channels [48*(k%4), +48).

v4 design (v2 refined). Measured SDMA facts that drove this shape:
  - each of the 16 SDMA engines streams at ~22.5 GB/s, with a ~50 ns
    per-descriptor floor -> descriptors must be >= ~2 KB to stay
    bytes-bound (a 512 B-descriptor scheme measured the same engine-busy
    as this one despite moving 30% fewer bytes);
  - a DRAM->DRAM copy cannot be combined with the diagonal scale, and a
    per-element diagonal scatter concentrates 4 B RMW descriptors on a
    few engines (v1: 470 us);
so the whole x -> out copy is routed through SBUF in [128 x 2048] f32
tiles (one channel each; partition p holds map rows {4p..4p+3}, i.e. an
8 KB CONTIGUOUS DRAM run per partition -> one 8 KB descriptor per
partition, maximizing DRAM burst locality), and the diagonal scale is
applied in SBUF. The diagonal elements sit at tile columns r*512+4p+r
(r = row-within-partition); a host-built 0/1 mask M0 marks them, and the
fix per channel is 2 DVE ops with PE-transposed attn (qs[p,r] =
attn[c,4p+r]-1, broadcast strides):
  F    = qs_b * M0          (nonzero only at the 4 diagonal positions)
  tile = (F + 1) * tile     (scalar_tensor_tensor, in place)

Pipelining: 20 tile slots (10 dedicated + 10 reclaimed from the e_b band
buffers once the band sums consume them). Per-slot load/store semaphores
are EXACT waits; shared-sem waits happen only at the FULL count of every
DMA that increments them (ein=64, din=80) - cumulative partial
thresholds race because a later DMA on the same ring can pre-increment
the semaphore while an earlier one still has a straggler descriptor.
"""

import numpy as np

B, C, S = 2, 192, 512
BW = 21          # band width
HALF = BW // 2   # 10
K = 7            # depthwise conv taps
CSH = C // 4     # 48 channels per core
N_CORES = 8

NDED = 9         # dedicated tile slots
NS = NDED + 10   # + 5 slots in et1 + 5 in et2 once band sums consume them
EBF = BW * S     # 10752 f32 per partition of e_b flat


def _dve_owns(c):
    # diagonal-fix work split 3:1 between DVE and GpSimd (the fix is
    # SBUF-bandwidth-bound; one engine alone paces the store stream, and
    # GpSimd's software ALU is slower per element)
    return c % 4 < 3

_prog = {}


def _build_program():
    """Raw-bass program (manual semaphores; Tile's multi-wait emission is
    rejected by this walrus).

    Engine plan:
      SP   - 48 tile loads x -> slot (per-slot lsem)
      ACT  - input DMAs, softmax exp/ln, 48 tile stores (per-slot ssem)
      DVE  - band sum, depthwise conv, softmax arith, per-channel diag fix
      PE   - 1x1 conv matmuls, 4 attn transposes

    vs milestones (DVE): 1=band sums done (et region free), 2=ct1, 3=ct2,
    4=sm+negmax, 5=ssum, 6=attn ready.
    psem (PE): 1=pointwise matmul, 2=transposes done.
    asem (ACT): 1=exp done, 2=rinv seed done.
    fsem (DVE): +1 per channel diagonal fix.
    ein: the 4 band-input DMAs (full count 64). din: the other 5 (full 80).
    """
    import concourse.bass as bass
    import concourse.mybir as mybir
    from concourse.ap import AP

    f32 = mybir.dt.float32
    Alu = mybir.AluOpType

    nc = bass.Bass()
    x_sh = nc.declare_dram_parameter("x_sh", [CSH, S, S], f32, isOutput=False)
    e_b = nc.declare_dram_parameter("e_b", [C, EBF], f32, isOutput=False)
    cw = nc.declare_dram_parameter("cw", [C, K], f32, isOutput=False)
    pwt = nc.declare_dram_parameter("pwt", [256, CSH], f32, isOutput=False)
    pb = nc.declare_dram_parameter("pb", [CSH, 1], f32, isOutput=False)
    i48 = nc.declare_dram_parameter("i48", [CSH, CSH], f32, isOutput=False)
    m0 = nc.declare_dram_parameter("m0", [128, 4 * S], f32, isOutput=False)
    out = nc.declare_dram_parameter("out", [CSH, S, S], f32, isOutput=True)

    # partition p <- map rows {4p..4p+3} (8 KB contiguous), free dims (r, w)
    x_re = x_sh.ap().rearrange("c (p r) w -> c p r w", p=128, r=4)
    out_re = out.ap().rearrange("c (p r) w -> c p r w", p=128, r=4)
    e_ap = e_b.ap()
    cw_ap = cw.ap()
    pwt_ap = pwt.ap()

    from contextlib import ExitStack

    with ExitStack() as ctx:
        ded = ctx.enter_context(nc.sbuf_tensor([128, NDED * 2048], f32))
        et1 = ctx.enter_context(nc.sbuf_tensor([128, EBF], f32))
        et2 = ctx.enter_context(nc.sbuf_tensor([128, EBF], f32))
        band1 = ctx.enter_context(nc.sbuf_tensor([128, S + K - 1], f32))
        band2 = ctx.enter_context(nc.sbuf_tensor([64, S + K - 1], f32))
        ct1 = ctx.enter_context(nc.sbuf_tensor([128, S], f32))
        ct2 = ctx.enter_context(nc.sbuf_tensor([128, S], f32))
        cw1 = ctx.enter_context(nc.sbuf_tensor([128, K], f32))
        cw2 = ctx.enter_context(nc.sbuf_tensor([64, K], f32))
        pw1 = ctx.enter_context(nc.sbuf_tensor([128, CSH], f32))
        pw2 = ctx.enter_context(nc.sbuf_tensor([128, CSH], f32))
        pbt = ctx.enter_context(nc.sbuf_tensor([CSH, 1], f32))
        sm = ctx.enter_context(nc.sbuf_tensor([CSH, S], f32))
        negmax = ctx.enter_context(nc.sbuf_tensor([CSH, 1], f32))
        ex = ctx.enter_context(nc.sbuf_tensor([CSH, S], f32))
        ssum = ctx.enter_context(nc.sbuf_tensor([CSH, 1], f32))
        rinv = ctx.enter_context(nc.sbuf_tensor([CSH, 1], f32))
        lse = ctx.enter_context(nc.sbuf_tensor([CSH, 1], f32))
        nrt = ctx.enter_context(nc.sbuf_tensor([CSH, 1], f32))
        attn = ctx.enter_context(nc.sbuf_tensor([CSH, S], f32))
        i48s = ctx.enter_context(nc.sbuf_tensor([CSH, CSH], f32))
        m0s = ctx.enter_context(nc.sbuf_tensor([128, 4 * S], f32))
        qm1 = ctx.enter_context(nc.sbuf_tensor([128, 4 * CSH], f32))
        fbuf = ctx.enter_context(nc.sbuf_tensor([128, 4 * S], f32))
        fbufp = ctx.enter_context(nc.sbuf_tensor([128, 4 * S], f32))
        ps = ctx.enter_context(nc.psum_tensor([CSH, S], f32))
        psq = ctx.enter_context(nc.psum_tensor([128, 4 * CSH], f32))
        ein = ctx.enter_context(nc.semaphore("ein"))
        din = ctx.enter_context(nc.semaphore("din"))
        vs = ctx.enter_context(nc.semaphore("vs"))
        psem = ctx.enter_context(nc.semaphore("psem"))
        asem = ctx.enter_context(nc.semaphore("asem"))
        fsemv = ctx.enter_context(nc.semaphore("fsemv"))
        fsemp = ctx.enter_context(nc.semaphore("fsemp"))
        lsem = [ctx.enter_context(nc.semaphore(f"ls{i}")) for i in range(NS)]
        ssem = [ctx.enter_context(nc.semaphore(f"ss{i}")) for i in range(NS)]
        block = ctx.enter_context(nc.Block())

        def slot_ap(c):
            s = c % NS
            if s < NDED:
                return ded.ap()[:, s * 2048 : (s + 1) * 2048]
            if s < NDED + 5:
                j = s - NDED
                return et1.ap()[:, j * 2048 : (j + 1) * 2048]
            j = s - NDED - 5
            return et2.ap()[:, j * 2048 : (j + 1) * 2048]

        @block.sync
        def _(sync):
            for c in range(CSH):
                s = c % NS
                if c == NDED:
                    sync.wait_ge(vs, 1)  # et1/et2 consumed by band sums
                if c >= NS:
                    sync.wait_ge(ssem[s], 16 * (c // NS))
                sync.dma_start(out=slot_ap(c), in_=x_re[c]).then_inc(lsem[s], 16)

        @block.scalar
        def _(scalar):
            scalar.dma_start(out=et1[:], in_=e_ap[0:128]).then_inc(ein, 16)
            scalar.dma_start(out=et2[0:64, :], in_=e_ap[128:C]).then_inc(ein, 16)
            scalar.dma_start(out=cw1[:], in_=cw_ap[0:128]).then_inc(ein, 16)
            scalar.dma_start(out=cw2[:], in_=cw_ap[128:C]).then_inc(ein, 16)
            scalar.dma_start(out=pw1[:], in_=pwt_ap[0:128]).then_inc(din, 16)
            scalar.dma_start(out=pw2[:], in_=pwt_ap[128:256]).then_inc(din, 16)
            scalar.dma_start(out=pbt[:], in_=pb.ap()).then_inc(din, 16)
            scalar.dma_start(out=i48s[:], in_=i48.ap()).then_inc(din, 16)
            scalar.dma_start(out=m0s[:], in_=m0.ap()).then_inc(din, 16)
            scalar.wait_ge(vs, 4)
            scalar.activation(
                out=ex[:], in_=sm[:], func=mybir.ActivationFunctionType.Exp,
                bias=negmax[:], scale=1.0,
            ).then_inc(asem, 1)
            # seed 1/ssum = exp(-ln(ssum)); DVE Newton-polishes it
            scalar.wait_ge(vs, 5)
            scalar.activation(
                out=lse[:], in_=ssum[:], func=mybir.ActivationFunctionType.Ln
            )
            scalar.activation(
                out=rinv[:], in_=lse[:], func=mybir.ActivationFunctionType.Exp,
                scale=-1.0,
            ).then_inc(asem, 1)
            for c in range(CSH):
                s = c % NS
                nv = sum(1 for c2 in range(c + 1) if _dve_owns(c2))
                if nv:
                    scalar.wait_ge(fsemv, nv)
                if c + 1 - nv:
                    scalar.wait_ge(fsemp, c + 1 - nv)
                scalar.dma_start(out=out_re[c], in_=slot_ap(c)).then_inc(
                    ssem[s], 16
                )
            # drain: the kernel must not end with store DMAs in flight
            for s in range(NS):
                n_stores = len(range(s, CSH, NS))
                scalar.wait_ge(ssem[s], 16 * n_stores)

        @block.vector
        def _(vector):
            vector.wait_ge(ein, 64)  # et1, et2, cw1, cw2 (full count: exact)
            # band sums over the 21 taps (mean's 1/21 folded into cw on host)
            for (band, et, p) in ((band1, et1, 128), (band2, et2, 64)):
                bs = band[0:p, 3 : 3 + S]
                vector.tensor_tensor(
                    out=bs, in0=et[0:p, 0:S], in1=et[0:p, S : 2 * S], op=Alu.add
                )
                for k in range(2, BW):
                    vector.tensor_tensor(
                        out=bs, in0=et[0:p, k * S : (k + 1) * S], in1=bs,
                        op=Alu.add,
                    )
            vector.memset(band1[:, 0:3], 0.0)
            vector.memset(band1[:, 3 + S :], 0.0)
            vector.memset(band2[:, 0:3], 0.0)
            vector.memset(band2[:, 3 + S :], 0.0)
            vector.memset(ct2[64:128, :], 0.0).then_inc(vs, 1)  # et region free
            # depthwise conv, 7 taps
            for (ct, band, cwt, p) in ((ct1, band1, cw1, 128), (ct2, band2, cw2, 64)):
                vector.tensor_scalar(
                    out=ct[0:p, :], in0=band[0:p, 0:S],
                    scalar1=cwt[0:p, 0:1], scalar2=None, op0=Alu.mult,
                )
                for t in range(1, K):
                    stt = vector.scalar_tensor_tensor(
                        out=ct[0:p, :], in0=band[0:p, t : t + S],
                        scalar=cwt[0:p, t : t + 1], in1=ct[0:p, :],
                        op0=Alu.mult, op1=Alu.add,
                    )
                stt.then_inc(vs, 1)  # vs=2 after ct1, vs=3 after ct2
            vector.wait_ge(psem, 1)
            vector.wait_ge(din, 80)  # pbt (and the rest; full count: exact)
            vector.tensor_scalar_add(out=sm[:], in0=ps[:], scalar1=pbt[:])
            vector.tensor_reduce(
                out=negmax[:], in_=sm[:], axis=mybir.AxisListType.X,
                op=Alu.max, negate=True,
            ).then_inc(vs, 1)  # vs=4: exp inputs ready
            vector.wait_ge(asem, 1)
            vector.tensor_reduce(
                out=ssum[:], in_=ex[:], axis=mybir.AxisListType.X, op=Alu.add
            ).then_inc(vs, 1)  # vs=5: ssum ready for ACT's 1/x seed
            vector.wait_ge(asem, 2)
            for _ in range(2):  # Newton: y <- y*(2 - x*y)
                vector.tensor_tensor(
                    out=nrt[:], in0=ssum[:], in1=rinv[:], op=Alu.mult
                )
                vector.tensor_scalar(
                    out=nrt[:], in0=nrt[:], scalar1=-1.0, scalar2=2.0,
                    op0=Alu.mult, op1=Alu.add,
                )
                vector.tensor_tensor(
                    out=rinv[:], in0=rinv[:], in1=nrt[:], op=Alu.mult
                )
            vector.tensor_scalar_mul(
                out=attn[:], in0=ex[:], scalar1=rinv[:]
            ).then_inc(vs, 1)  # vs=6: attn ready for PE transposes
            vector.wait_ge(psem, 2)
            vector.tensor_scalar_add(
                out=qm1[:], in0=psq[:], scalar1=-1.0
            ).then_inc(vs, 1)  # vs=7: qm1 ready (GpSimd fix loop waits this)
            m0v = AP(
                tensor=m0s.ap().tensor, offset=m0s.ap().offset,
                ap=[list(m0s.ap().ap[0]), [S, 4], [1, S]],
            )
            fb = fbuf.ap()
            fb4 = AP(
                tensor=fb.tensor, offset=fb.offset,
                ap=[list(fb.ap[0]), [S, 4], [1, S]],
            )
            for c in range(CSH):
                if not _dve_owns(c):
                    continue
                s = c % NS
                vector.wait_ge(lsem[s], 16 * (c // NS + 1))
                tile = slot_ap(c)
                t4 = AP(
                    tensor=tile.tensor, offset=tile.offset,
                    ap=[list(tile.ap[0]), [S, 4], [1, S]],
                )
                qsb = (
                    qm1.ap()[:, c : 4 * CSH : CSH]
                    .unsqueeze(2)
                    .to_broadcast([128, 4, S])
                )
                # F = qs*M0 (nonzero only at diag positions); tile = (F+1)*tile
                vector.tensor_tensor(out=fb4, in0=qsb, in1=m0v, op=Alu.mult)
                vector.scalar_tensor_tensor(
                    out=t4, in0=fb4, scalar=1.0, in1=t4,
                    op0=Alu.add, op1=Alu.mult,
                ).then_inc(fsemv, 1)

        @block.gpsimd
        def _(gpsimd):
            gpsimd.wait_ge(vs, 7)  # qm1 (and transitively m0s, attn) ready
            m0v = AP(
                tensor=m0s.ap().tensor, offset=m0s.ap().offset,
                ap=[list(m0s.ap().ap[0]), [S, 4], [1, S]],
            )
            fb = fbufp.ap()
            fb4 = AP(
                tensor=fb.tensor, offset=fb.offset,
                ap=[list(fb.ap[0]), [S, 4], [1, S]],
            )
            for c in range(CSH):
                if _dve_owns(c):
                    continue
                s = c % NS
                gpsimd.wait_ge(lsem[s], 16 * (c // NS + 1))
                tile = slot_ap(c)
                t4 = AP(
                    tensor=tile.tensor, offset=tile.offset,
                    ap=[list(tile.ap[0]), [S, 4], [1, S]],
                )
                qsb = (
                    qm1.ap()[:, c : 4 * CSH : CSH]
                    .unsqueeze(2)
                    .to_broadcast([128, 4, S])
                )
                # Pool ISA lacks scalar_tensor_tensor; 3 plain TT ops instead
                gpsimd.tensor_tensor(out=fb4, in0=qsb, in1=m0v, op=Alu.mult)
                gpsimd.tensor_tensor(out=fb4, in0=fb4, in1=t4, op=Alu.mult)
                gpsimd.tensor_tensor(
                    out=t4, in0=t4, in1=fb4, op=Alu.add
                ).then_inc(fsemp, 1)

        @block.tensor
        def _(tensor):
            tensor.wait_ge(din, 80)
            tensor.wait_ge(vs, 3)
            nc.tensor.matmul(ps[:], lhsT=pw1[:], rhs=ct1[:], start=True, stop=False)
            nc.tensor.matmul(
                ps[:], lhsT=pw2[:], rhs=ct2[:], start=False, stop=True
            ).then_inc(psem, 1)
            tensor.wait_ge(vs, 6)
            # qm1[p, r*48+c] <- attn[c, 4p+r]: transpose the stride-4 slices
            for r in range(4):
                mm = nc.tensor.matmul(
                    psq[:, r * CSH : (r + 1) * CSH],
                    lhsT=attn[:, r : S : 4],
                    rhs=i48s[:],
                    start=True, stop=True,
                )
            mm.then_inc(psem, 1)  # psem=2: all transposes done

    return nc


def _get_program():
    if "p" not in _prog:
        _prog["p"] = _build_program()
    return _prog["p"]


def _host_prep(x, conv_w, point_w, point_b):
    """Build per-core input maps. Everything here is slicing/layout only."""
    x = np.asarray(x, dtype=np.float32)
    conv_w = np.asarray(conv_w, dtype=np.float32)
    point_w = np.asarray(point_w, dtype=np.float32)
    point_b = np.asarray(point_b, dtype=np.float32)

    # E[b,c,k,j] = xpad[b,c,j+k,j]  (rows padded by HALF), via diagonal views
    E = np.zeros((B, C, BW, S), dtype=np.float32)
    for k in range(BW):
        o = HALF - k
        d = np.diagonal(x, offset=o, axis1=2, axis2=3)
        if o >= 0:
            E[:, :, k, o:S] = d
        else:
            E[:, :, k, 0 : S + o] = d
    E = E.reshape(B, C, EBF)

    cw_all = np.ascontiguousarray(conv_w.reshape(C, K) / np.float32(BW))
    eye48 = np.eye(CSH, dtype=np.float32)
    # mask of diagonal positions in the [128, 4*S] tile layout:
    # partition p holds map rows 4p+r; row 4p+r's diagonal is at column 4p+r
    m0 = np.zeros((128, 4 * S), dtype=np.float32)
    for p in range(128):
        for r in range(4):
            m0[p, r * S + 4 * p + r] = 1.0

    in_maps = []
    for core in range(N_CORES):
        b, cb = divmod(core, 4)
        c0 = cb * CSH
        pwt_sh = np.zeros((256, CSH), dtype=np.float32)
        pwt_sh[:C] = point_w[c0 : c0 + CSH, :].T
        in_maps.append(
            {
                "x_sh": np.ascontiguousarray(x[b, c0 : c0 + CSH]),
                "e_b": np.ascontiguousarray(E[b]),
                "cw": cw_all,
                "pwt": pwt_sh,
                "pb": np.ascontiguousarray(point_b[c0 : c0 + CSH].reshape(CSH, 1)),
                "i48": eye48,
                "m0": m0,
            }
        )
    return in_maps


def _run(inputs, trace=False):
    from concourse.bass_utils import run_bass_kernel_spmd

    nc = _get_program()
    in_maps = _host_prep(**inputs)
    res = run_bass_kernel_spmd(
        nc, in_maps, core_ids=list(range(N_CORES)), trace=trace
    )
    out = np.empty((B, C, S, S), dtype=np.float32)
    for core in range(N_CORES):
        b, cb = divmod(core, 4)
        c0 = cb * CSH
        out[b, c0 : c0 + CSH] = res.results[core]["out"]
    return out, res


def kernel(x, conv_w, point_w, point_b):
    out, _ = _run(dict(x=x, conv_w=conv_w, point_w=point_w, point_b=point_b))
    return out


# revision 38
# speedup vs baseline: 1.1839x; 1.1839x over previous
"""DiagonalBandAttention Trainium2 kernel.

Computation (reference semantics):
  band[b,c,j]  = mean_{k=0..20} xpad[b,c,j+k,j]        (rows zero-padded by 10)
  conv[b,c,s]  = depthwise_conv1d(band, conv_w, k=7, pad=3)   (cross-correlation)
  attn[b,d,s]  = softmax_s( sum_c point_w[d,c]*conv[b,c,s] + point_b[d] )
  out          = x, with out[b,c,j,j] = x[b,c,j,j] * attn[b,c,j]

Output is x copied verbatim except the main diagonal of each [S,S] map.
Sharding (8 cores): core k handles batch b = k//4, channels [48*(k%4), +48).

v4 design (v2 refined). Measured SDMA facts that drove this shape:
  - each of the 16 SDMA engines streams at ~22.5 GB/s, with a ~50 ns
    per-descriptor floor -> descriptors must be >= ~2 KB to stay
    bytes-bound (a 512 B-descriptor scheme measured the same engine-busy
    as this one despite moving 30% fewer bytes);
  - a DRAM->DRAM copy cannot be combined with the diagonal scale, and a
    per-element diagonal scatter concentrates 4 B RMW descriptors on a
    few engines (v1: 470 us);
so the whole x -> out copy is routed through SBUF in [128 x 2048] f32
tiles (one channel each; partition p holds map rows {4p..4p+3}, i.e. an
8 KB CONTIGUOUS DRAM run per partition -> one 8 KB descriptor per
partition, maximizing DRAM burst locality), and the diagonal scale is
applied in SBUF. The diagonal elements sit at tile columns r*512+4p+r
(r = row-within-partition). The columns that can hold a diagonal form the
regular stride-4 set {513r + 4q}, and within that [128, 4, 128] strided
view the diagonal is identity-aligned (partition p's element at q=p), so
the fix per channel is 2 small DVE ops (8x less SBUF traffic than a
full-tile masked multiply, which left DVE co-critical at 248us busy):
  G4[p,r,q]      = qs[p,r] * I128[p,q]        (broadcast strides)
  tile_diag[...] = (G4 + 1) * tile_diag       (scalar_tensor_tensor)
with qs[p,r] = attn[c,4p+r]-1 from 4 PE transposes.

Pipelining: 21 tile slots (11 dedicated + 10 reclaimed from the e_b band
buffers once the band sums consume them). Per-slot load/store semaphores
are EXACT waits; shared-sem waits happen only at the FULL count of every
DMA that increments them (ein=64, din=80) - cumulative partial
thresholds race because a later DMA on the same ring can pre-increment
the semaphore while an earlier one still has a straggler descriptor.
"""

import numpy as np

B, C, S = 2, 192, 512
BW = 21          # band width
HALF = BW // 2   # 10
K = 7            # depthwise conv taps
CSH = C // 4     # 48 channels per core
N_CORES = 8

NDED = 11        # dedicated tile slots
NS = NDED + 10   # + 5 slots in et1 + 5 in et2 once band sums consume them
EBF = BW * S     # 10752 f32 per partition of e_b flat

_prog = {}


def _build_program():
    """Raw-bass program (manual semaphores; Tile's multi-wait emission is
    rejected by this walrus).

    Engine plan:
      SP   - 48 tile loads x -> slot (per-slot lsem)
      ACT  - input DMAs, softmax exp/ln, 48 tile stores (per-slot ssem)
      DVE  - band sum, depthwise conv, softmax arith, per-channel diag fix
      PE   - 1x1 conv matmuls, 4 attn transposes

    vs milestones (DVE): 1=band sums done (et region free), 2=ct1, 3=ct2,
    4=sm+negmax, 5=ssum, 6=attn ready.
    psem (PE): 1=pointwise matmul, 2=transposes done.
    asem (ACT): 1=exp done, 2=rinv seed done.
    fsem (DVE): +1 per channel diagonal fix.
    ein: the 4 band-input DMAs (full count 64). din: the other 5 (full 80).
    """
    import concourse.bass as bass
    import concourse.mybir as mybir
    from concourse.ap import AP

    f32 = mybir.dt.float32
    Alu = mybir.AluOpType

    nc = bass.Bass()
    x_sh = nc.declare_dram_parameter("x_sh", [CSH, S, S], f32, isOutput=False)
    e_b = nc.declare_dram_parameter("e_b", [C, EBF], f32, isOutput=False)
    cw = nc.declare_dram_parameter("cw", [C, K], f32, isOutput=False)
    pwt = nc.declare_dram_parameter("pwt", [256, CSH], f32, isOutput=False)
    pb = nc.declare_dram_parameter("pb", [CSH, 1], f32, isOutput=False)
    i48 = nc.declare_dram_parameter("i48", [CSH, CSH], f32, isOutput=False)
    i128 = nc.declare_dram_parameter("i128", [128, 128], f32, isOutput=False)
    out = nc.declare_dram_parameter("out", [CSH, S, S], f32, isOutput=True)

    # partition p <- map rows {4p..4p+3} (8 KB contiguous), free dims (r, w)
    x_re = x_sh.ap().rearrange("c (p r) w -> c p r w", p=128, r=4)
    out_re = out.ap().rearrange("c (p r) w -> c p r w", p=128, r=4)
    e_ap = e_b.ap()
    cw_ap = cw.ap()
    pwt_ap = pwt.ap()

    from contextlib import ExitStack

    with ExitStack() as ctx:
        ded = ctx.enter_context(nc.sbuf_tensor([128, NDED * 2048], f32))
        et1 = ctx.enter_context(nc.sbuf_tensor([128, EBF], f32))
        et2 = ctx.enter_context(nc.sbuf_tensor([128, EBF], f32))
        band1 = ctx.enter_context(nc.sbuf_tensor([128, S + K - 1], f32))
        band2 = ctx.enter_context(nc.sbuf_tensor([64, S + K - 1], f32))
        ct1 = ctx.enter_context(nc.sbuf_tensor([128, S], f32))
        ct2 = ctx.enter_context(nc.sbuf_tensor([128, S], f32))
        cw1 = ctx.enter_context(nc.sbuf_tensor([128, K], f32))
        cw2 = ctx.enter_context(nc.sbuf_tensor([64, K], f32))
        pw1 = ctx.enter_context(nc.sbuf_tensor([128, CSH], f32))
        pw2 = ctx.enter_context(nc.sbuf_tensor([128, CSH], f32))
        pbt = ctx.enter_context(nc.sbuf_tensor([CSH, 1], f32))
        sm = ctx.enter_context(nc.sbuf_tensor([CSH, S], f32))
        negmax = ctx.enter_context(nc.sbuf_tensor([CSH, 1], f32))
        ex = ctx.enter_context(nc.sbuf_tensor([CSH, S], f32))
        ssum = ctx.enter_context(nc.sbuf_tensor([CSH, 1], f32))
        rinv = ctx.enter_context(nc.sbuf_tensor([CSH, 1], f32))
        lse = ctx.enter_context(nc.sbuf_tensor([CSH, 1], f32))
        nrt = ctx.enter_context(nc.sbuf_tensor([CSH, 1], f32))
        attn = ctx.enter_context(nc.sbuf_tensor([CSH, S], f32))
        i48s = ctx.enter_context(nc.sbuf_tensor([CSH, CSH], f32))
        i128s = ctx.enter_context(nc.sbuf_tensor([128, 128], f32))
        qm1 = ctx.enter_context(nc.sbuf_tensor([128, 4 * CSH], f32))
        fbuf = ctx.enter_context(nc.sbuf_tensor([128, S], f32))
        ps = ctx.enter_context(nc.psum_tensor([CSH, S], f32))
        psq = ctx.enter_context(nc.psum_tensor([128, 4 * CSH], f32))
        ein = ctx.enter_context(nc.semaphore("ein"))
        din = ctx.enter_context(nc.semaphore("din"))
        vs = ctx.enter_context(nc.semaphore("vs"))
        psem = ctx.enter_context(nc.semaphore("psem"))
        asem = ctx.enter_context(nc.semaphore("asem"))
        fsem = ctx.enter_context(nc.semaphore("fsem"))
        lsem = [ctx.enter_context(nc.semaphore(f"ls{i}")) for i in range(NS)]
        ssem = [ctx.enter_context(nc.semaphore(f"ss{i}")) for i in range(NS)]
        block = ctx.enter_context(nc.Block())

        def slot_ap(c):
            s = c % NS
            if s < NDED:
                return ded.ap()[:, s * 2048 : (s + 1) * 2048]
            if s < NDED + 5:
                j = s - NDED
                return et1.ap()[:, j * 2048 : (j + 1) * 2048]
            j = s - NDED - 5
            return et2.ap()[:, j * 2048 : (j + 1) * 2048]

        @block.sync
        def _(sync):
            # et2 rides the otherwise-idle SP ring so both e_b halves land
            # concurrently and the softmax path starts earlier
            sync.dma_start(out=et2[0:64, :], in_=e_ap[128:C]).then_inc(ein, 16)
            for c in range(CSH):
                s = c % NS
                if c == NDED:
                    sync.wait_ge(vs, 1)  # et1/et2 consumed by band sums
                if c >= NS:
                    sync.wait_ge(ssem[s], 16 * (c // NS))
                sync.dma_start(out=slot_ap(c), in_=x_re[c]).then_inc(lsem[s], 16)

        @block.scalar
        def _(scalar):
            scalar.dma_start(out=et1[:], in_=e_ap[0:128]).then_inc(ein, 16)
            scalar.dma_start(out=cw1[:], in_=cw_ap[0:128]).then_inc(ein, 16)
            scalar.dma_start(out=cw2[:], in_=cw_ap[128:C]).then_inc(ein, 16)
            scalar.dma_start(out=pw1[:], in_=pwt_ap[0:128]).then_inc(din, 16)
            scalar.dma_start(out=pw2[:], in_=pwt_ap[128:256]).then_inc(din, 16)
            scalar.dma_start(out=pbt[:], in_=pb.ap()).then_inc(din, 16)
            scalar.dma_start(out=i48s[:], in_=i48.ap()).then_inc(din, 16)
            scalar.dma_start(out=i128s[:], in_=i128.ap()).then_inc(din, 16)
            scalar.wait_ge(vs, 4)
            scalar.activation(
                out=ex[:], in_=sm[:], func=mybir.ActivationFunctionType.Exp,
                bias=negmax[:], scale=1.0,
            ).then_inc(asem, 1)
            # seed 1/ssum = exp(-ln(ssum)); DVE Newton-polishes it
            scalar.wait_ge(vs, 5)
            scalar.activation(
                out=lse[:], in_=ssum[:], func=mybir.ActivationFunctionType.Ln
            )
            scalar.activation(
                out=rinv[:], in_=lse[:], func=mybir.ActivationFunctionType.Exp,
                scale=-1.0,
            ).then_inc(asem, 1)
            for c in range(CSH):
                s = c % NS
                scalar.wait_ge(fsem, c + 1)
                scalar.dma_start(out=out_re[c], in_=slot_ap(c)).then_inc(
                    ssem[s], 16
                )
            # drain: the kernel must not end with store DMAs in flight
            for s in range(NS):
                n_stores = len(range(s, CSH, NS))
                scalar.wait_ge(ssem[s], 16 * n_stores)

        @block.vector
        def _(vector):
            vector.wait_ge(ein, 64)  # et1, et2, cw1, cw2 (full count: exact)
            # band sums over the 21 taps (mean's 1/21 folded into cw on host)
            for (band, et, p) in ((band1, et1, 128), (band2, et2, 64)):
                bs = band[0:p, 3 : 3 + S]
                vector.tensor_tensor(
                    out=bs, in0=et[0:p, 0:S], in1=et[0:p, S : 2 * S], op=Alu.add
                )
                for k in range(2, BW):
                    vector.tensor_tensor(
                        out=bs, in0=et[0:p, k * S : (k + 1) * S], in1=bs,
                        op=Alu.add,
                    )
            vector.memset(band1[:, 0:3], 0.0)
            vector.memset(band1[:, 3 + S :], 0.0)
            vector.memset(band2[:, 0:3], 0.0)
            vector.memset(band2[:, 3 + S :], 0.0)
            vector.memset(ct2[64:128, :], 0.0).then_inc(vs, 1)  # et region free
            # depthwise conv, 7 taps
            for (ct, band, cwt, p) in ((ct1, band1, cw1, 128), (ct2, band2, cw2, 64)):
                vector.tensor_scalar(
                    out=ct[0:p, :], in0=band[0:p, 0:S],
                    scalar1=cwt[0:p, 0:1], scalar2=None, op0=Alu.mult,
                )
                for t in range(1, K):
                    stt = vector.scalar_tensor_tensor(
                        out=ct[0:p, :], in0=band[0:p, t : t + S],
                        scalar=cwt[0:p, t : t + 1], in1=ct[0:p, :],
                        op0=Alu.mult, op1=Alu.add,
                    )
                stt.then_inc(vs, 1)  # vs=2 after ct1, vs=3 after ct2
            vector.wait_ge(psem, 1)
            vector.wait_ge(din, 80)  # pbt (and the rest; full count: exact)
            vector.tensor_scalar_add(out=sm[:], in0=ps[:], scalar1=pbt[:])
            vector.tensor_reduce(
                out=negmax[:], in_=sm[:], axis=mybir.AxisListType.X,
                op=Alu.max, negate=True,
            ).then_inc(vs, 1)  # vs=4: exp inputs ready
            vector.wait_ge(asem, 1)
            vector.tensor_reduce(
                out=ssum[:], in_=ex[:], axis=mybir.AxisListType.X, op=Alu.add
            ).then_inc(vs, 1)  # vs=5: ssum ready for ACT's 1/x seed
            vector.wait_ge(asem, 2)
            for _ in range(2):  # Newton: y <- y*(2 - x*y)
                vector.tensor_tensor(
                    out=nrt[:], in0=ssum[:], in1=rinv[:], op=Alu.mult
                )
                vector.tensor_scalar(
                    out=nrt[:], in0=nrt[:], scalar1=-1.0, scalar2=2.0,
                    op0=Alu.mult, op1=Alu.add,
                )
                vector.tensor_tensor(
                    out=rinv[:], in0=rinv[:], in1=nrt[:], op=Alu.mult
                )
            vector.tensor_scalar_mul(
                out=attn[:], in0=ex[:], scalar1=rinv[:]
            ).then_inc(vs, 1)  # vs=6: attn ready for PE transposes
            vector.wait_ge(psem, 2)
            vector.tensor_scalar_add(out=qm1[:], in0=psq[:], scalar1=-1.0)
            i128b = i128s.ap().unsqueeze(1).to_broadcast([128, 4, 128])
            fb = fbuf.ap()
            g4 = AP(
                tensor=fb.tensor, offset=fb.offset,
                ap=[list(fb.ap[0]), [128, 4], [1, 128]],
            )
            for c in range(CSH):
                s = c % NS
                vector.wait_ge(lsem[s], 16 * (c // NS + 1))
                tile = slot_ap(c)
                # diagonal-bearing columns: 513r + 4q, identity-aligned (q=p)
                t4d = AP(
                    tensor=tile.tensor, offset=tile.offset,
                    ap=[list(tile.ap[0]), [S + 1, 4], [4, 128]],
                )
                qsb = (
                    qm1.ap()[:, c : 4 * CSH : CSH]
                    .unsqueeze(2)
                    .to_broadcast([128, 4, 128])
                )
                # G4 = qs*I128 (diag = attn-1, else 0); tile_d = (G4+1)*tile_d
                vector.tensor_tensor(out=g4, in0=qsb, in1=i128b, op=Alu.mult)
                vector.scalar_tensor_tensor(
                    out=t4d, in0=g4, scalar=1.0, in1=t4d,
                    op0=Alu.add, op1=Alu.mult,
                ).then_inc(fsem, 1)

        @block.tensor
        def _(tensor):
            tensor.wait_ge(din, 80)
            tensor.wait_ge(vs, 3)
            nc.tensor.matmul(ps[:], lhsT=pw1[:], rhs=ct1[:], start=True, stop=False)
            nc.tensor.matmul(
                ps[:], lhsT=pw2[:], rhs=ct2[:], start=False, stop=True
            ).then_inc(psem, 1)
            tensor.wait_ge(vs, 6)
            # qm1[p, r*48+c] <- attn[c, 4p+r]: transpose the stride-4 slices
            for r in range(4):
                mm = nc.tensor.matmul(
                    psq[:, r * CSH : (r + 1) * CSH],
                    lhsT=attn[:, r : S : 4],
                    rhs=i48s[:],
                    start=True, stop=True,
                )
            mm.then_inc(psem, 1)  # psem=2: all transposes done

    return nc


def _get_program():
    if "p" not in _prog:
        _prog["p"] = _build_program()
    return _prog["p"]


def _host_prep(x, conv_w, point_w, point_b):
    """Build per-core input maps. Everything here is slicing/layout only."""
    x = np.asarray(x, dtype=np.float32)
    conv_w = np.asarray(conv_w, dtype=np.float32)
    point_w = np.asarray(point_w, dtype=np.float32)
    point_b = np.asarray(point_b, dtype=np.float32)

    # E[b,c,k,j] = xpad[b,c,j+k,j]  (rows padded by HALF), via diagonal views
    E = np.zeros((B, C, BW, S), dtype=np.float32)
    for k in range(BW):
        o = HALF - k
        d = np.diagonal(x, offset=o, axis1=2, axis2=3)
        if o >= 0:
            E[:, :, k, o:S] = d
        else:
            E[:, :, k, 0 : S + o] = d
    E = E.reshape(B, C, EBF)

    cw_all = np.ascontiguousarray(conv_w.reshape(C, K) / np.float32(BW))
    eye48 = np.eye(CSH, dtype=np.float32)
    eye128 = np.eye(128, dtype=np.float32)

    in_maps = []
    for core in range(N_CORES):
        b, cb = divmod(core, 4)
        c0 = cb * CSH
        pwt_sh = np.zeros((256, CSH), dtype=np.float32)
        pwt_sh[:C] = point_w[c0 : c0 + CSH, :].T
        in_maps.append(
            {
                "x_sh": np.ascontiguousarray(x[b, c0 : c0 + CSH]),
                "e_b": np.ascontiguousarray(E[b]),
                "cw": cw_all,
                "pwt": pwt_sh,
                "pb": np.ascontiguousarray(point_b[c0 : c0 + CSH].reshape(CSH, 1)),
                "i48": eye48,
                "i128": eye128,
            }
        )
    return in_maps


def _run(inputs, trace=False):
    from concourse.bass_utils import run_bass_kernel_spmd

    nc = _get_program()
    in_maps = _host_prep(**inputs)
    res = run_bass_kernel_spmd(
        nc, in_maps, core_ids=list(range(N_CORES)), trace=trace
    )
    out = np.empty((B, C, S, S), dtype=np.float32)
    for core in range(N_CORES):
        b, cb = divmod(core, 4)
        c0 = cb * CSH
        out[b, c0 : c0 + CSH] = res.results[core]["out"]
    return out, res


def kernel(x, conv_w, point_w, point_b):
    out, _ = _run(dict(x=x, conv_w=conv_w, point_w=point_w, point_b=point_b))
    return out


# revision 42
# speedup vs baseline: 1.2571x; 1.0618x over previous
"""DiagonalBandAttention Trainium2 kernel.

Computation (reference semantics):
  band[b,c,j]  = mean_{k=0..20} xpad[b,c,j+k,j]        (rows zero-padded by 10)
  conv[b,c,s]  = depthwise_conv1d(band, conv_w, k=7, pad=3)   (cross-correlation)
  attn[b,d,s]  = softmax_s( sum_c point_w[d,c]*conv[b,c,s] + point_b[d] )
  out          = x, with out[b,c,j,j] = x[b,c,j,j] * attn[b,c,j]

Output is x copied verbatim except the main diagonal of each [S,S] map.
Sharding (8 cores): core k handles batch b = k//4, channels [48*(k%4), +48).

v4 design (v2 refined). Measured SDMA facts that drove this shape:
  - each of the 16 SDMA engines streams at ~22.5 GB/s, with a ~50 ns
    per-descriptor floor -> descriptors must be >= ~2 KB to stay
    bytes-bound (a 512 B-descriptor scheme measured the same engine-busy
    as this one despite moving 30% fewer bytes);
  - a DRAM->DRAM copy cannot be combined with the diagonal scale, and a
    per-element diagonal scatter concentrates 4 B RMW descriptors on a
    few engines (v1: 470 us);
so the whole x -> out copy is routed through SBUF in [128 x 2048] f32
tiles (one channel each; partition p holds map rows {4p..4p+3}, i.e. an
8 KB CONTIGUOUS DRAM run per partition -> one 8 KB descriptor per
partition, maximizing DRAM burst locality), and the diagonal scale is
applied in SBUF. The diagonal elements sit at tile columns r*512+4p+r
(r = row-within-partition); a host-built 0/1 mask M0 marks them, and the
fix per channel is 2 DVE ops with PE-transposed attn (qs[p,r] =
attn[c,4p+r]-1, broadcast strides):
  F    = qs_b * M0          (nonzero only at the 4 diagonal positions)
  tile = (F + 1) * tile     (scalar_tensor_tensor, in place)

Pipelining: 20 tile slots (10 dedicated + 10 reclaimed from the e_b band
buffers once the band sums consume them). Per-slot load/store semaphores
are EXACT waits; shared-sem waits happen only at the FULL count of every
DMA that increments them (ein=64, din=80) - cumulative partial
thresholds race because a later DMA on the same ring can pre-increment
the semaphore while an earlier one still has a straggler descriptor.
"""

import numpy as np

B, C, S = 2, 192, 512
BW = 21          # band width
HALF = BW // 2   # 10
K = 7            # depthwise conv taps
CSH = C // 4     # 48 channels per core
N_CORES = 8

NDED = 10        # dedicated tile slots
NS = NDED + 10   # + 5 slots in et1 + 5 in et2 once band sums consume them
EBF = BW * S     # 10752 f32 per partition of e_b flat

_prog = {}


def _build_program():
    """Raw-bass program (manual semaphores; Tile's multi-wait emission is
    rejected by this walrus).

    Engine plan:
      SP   - 48 tile loads x -> slot (per-slot lsem)
      ACT  - input DMAs, softmax exp/ln, 48 tile stores (per-slot ssem)
      DVE  - band sum, depthwise conv, softmax arith, per-channel diag fix
      PE   - 1x1 conv matmuls, 4 attn transposes

    vs milestones (DVE): 1=band sums done (et region free), 2=ct1, 3=ct2,
    4=sm+negmax, 5=ssum, 6=attn ready.
    psem (PE): 1=pointwise matmul, 2=transposes done.
    asem (ACT): 1=exp done, 2=rinv seed done.
    fsem (DVE): +1 per channel diagonal fix.
    ein: the 4 band-input DMAs (full count 64). din: the other 5 (full 80).
    """
    import concourse.bass as bass
    import concourse.mybir as mybir
    from concourse.ap import AP

    f32 = mybir.dt.float32
    Alu = mybir.AluOpType

    nc = bass.Bass()
    x_sh = nc.declare_dram_parameter("x_sh", [CSH, S, S], f32, isOutput=False)
    e_b = nc.declare_dram_parameter("e_b", [C, EBF], f32, isOutput=False)
    cw = nc.declare_dram_parameter("cw", [C, K], f32, isOutput=False)
    pwt = nc.declare_dram_parameter("pwt", [256, CSH], f32, isOutput=False)
    pb = nc.declare_dram_parameter("pb", [CSH, 1], f32, isOutput=False)
    i48 = nc.declare_dram_parameter("i48", [CSH, CSH], f32, isOutput=False)
    m0 = nc.declare_dram_parameter("m0", [128, 4 * S], f32, isOutput=False)
    out = nc.declare_dram_parameter("out", [CSH, S, S], f32, isOutput=True)

    # partition p <- map rows {4p..4p+3} (8 KB contiguous), free dims (r, w)
    x_re = x_sh.ap().rearrange("c (p r) w -> c p r w", p=128, r=4)
    out_re = out.ap().rearrange("c (p r) w -> c p r w", p=128, r=4)
    e_ap = e_b.ap()
    cw_ap = cw.ap()
    pwt_ap = pwt.ap()

    from contextlib import ExitStack

    with ExitStack() as ctx:
        ded = ctx.enter_context(nc.sbuf_tensor([128, NDED * 2048], f32))
        et1 = ctx.enter_context(nc.sbuf_tensor([128, EBF], f32))
        et2 = ctx.enter_context(nc.sbuf_tensor([128, EBF], f32))
        band1 = ctx.enter_context(nc.sbuf_tensor([128, S + K - 1], f32))
        band2 = ctx.enter_context(nc.sbuf_tensor([64, S + K - 1], f32))
        ct1 = ctx.enter_context(nc.sbuf_tensor([128, S], f32))
        ct2 = ctx.enter_context(nc.sbuf_tensor([128, S], f32))
        cw1 = ctx.enter_context(nc.sbuf_tensor([128, K], f32))
        cw2 = ctx.enter_context(nc.sbuf_tensor([64, K], f32))
        pw1 = ctx.enter_context(nc.sbuf_tensor([128, CSH], f32))
        pw2 = ctx.enter_context(nc.sbuf_tensor([128, CSH], f32))
        pbt = ctx.enter_context(nc.sbuf_tensor([CSH, 1], f32))
        sm = ctx.enter_context(nc.sbuf_tensor([CSH, S], f32))
        negmax = ctx.enter_context(nc.sbuf_tensor([CSH, 1], f32))
        ex = ctx.enter_context(nc.sbuf_tensor([CSH, S], f32))
        ssum = ctx.enter_context(nc.sbuf_tensor([CSH, 1], f32))
        rinv = ctx.enter_context(nc.sbuf_tensor([CSH, 1], f32))
        lse = ctx.enter_context(nc.sbuf_tensor([CSH, 1], f32))
        nrt = ctx.enter_context(nc.sbuf_tensor([CSH, 1], f32))
        attn = ctx.enter_context(nc.sbuf_tensor([CSH, S], f32))
        i48s = ctx.enter_context(nc.sbuf_tensor([CSH, CSH], f32))
        m0s = ctx.enter_context(nc.sbuf_tensor([128, 4 * S], f32))
        qm1 = ctx.enter_context(nc.sbuf_tensor([128, 4 * CSH], f32))
        fbuf = ctx.enter_context(nc.sbuf_tensor([128, 4 * S], f32))
        ps = ctx.enter_context(nc.psum_tensor([CSH, S], f32))
        psq = ctx.enter_context(nc.psum_tensor([128, 4 * CSH], f32))
        ein = ctx.enter_context(nc.semaphore("ein"))
        din = ctx.enter_context(nc.semaphore("din"))
        vs = ctx.enter_context(nc.semaphore("vs"))
        psem = ctx.enter_context(nc.semaphore("psem"))
        asem = ctx.enter_context(nc.semaphore("asem"))
        fsem = ctx.enter_context(nc.semaphore("fsem"))
        lsem = [ctx.enter_context(nc.semaphore(f"ls{i}")) for i in range(NS)]
        ssem = [ctx.enter_context(nc.semaphore(f"ss{i}")) for i in range(NS)]
        block = ctx.enter_context(nc.Block())

        def slot_ap(c):
            s = c % NS
            if s < NDED:
                return ded.ap()[:, s * 2048 : (s + 1) * 2048]
            if s < NDED + 5:
                j = s - NDED
                return et1.ap()[:, j * 2048 : (j + 1) * 2048]
            j = s - NDED - 5
            return et2.ap()[:, j * 2048 : (j + 1) * 2048]

        @block.sync
        def _(sync):
            # et2 rides the otherwise-idle SP ring so both e_b halves land
            # concurrently and the softmax path starts earlier
            sync.dma_start(out=et2[0:64, :], in_=e_ap[128:C]).then_inc(ein, 16)
            for c in range(CSH):
                s = c % NS
                if c == NDED:
                    sync.wait_ge(vs, 1)  # et1/et2 consumed by band sums
                if c >= NS:
                    sync.wait_ge(ssem[s], 16 * (c // NS))
                sync.dma_start(out=slot_ap(c), in_=x_re[c]).then_inc(lsem[s], 16)

        @block.scalar
        def _(scalar):
            scalar.dma_start(out=et1[:], in_=e_ap[0:128]).then_inc(ein, 16)
            scalar.dma_start(out=cw1[:], in_=cw_ap[0:128]).then_inc(ein, 16)
            scalar.dma_start(out=cw2[:], in_=cw_ap[128:C]).then_inc(ein, 16)
            scalar.dma_start(out=pw1[:], in_=pwt_ap[0:128]).then_inc(din, 16)
            scalar.dma_start(out=pw2[:], in_=pwt_ap[128:256]).then_inc(din, 16)
            scalar.dma_start(out=pbt[:], in_=pb.ap()).then_inc(din, 16)
            scalar.dma_start(out=i48s[:], in_=i48.ap()).then_inc(din, 16)
            scalar.dma_start(out=m0s[:], in_=m0.ap()).then_inc(din, 16)
            scalar.wait_ge(vs, 4)
            scalar.activation(
                out=ex[:], in_=sm[:], func=mybir.ActivationFunctionType.Exp,
                bias=negmax[:], scale=1.0,
            ).then_inc(asem, 1)
            # seed 1/ssum = exp(-ln(ssum)); DVE Newton-polishes it
            scalar.wait_ge(vs, 5)
            scalar.activation(
                out=lse[:], in_=ssum[:], func=mybir.ActivationFunctionType.Ln
            )
            scalar.activation(
                out=rinv[:], in_=lse[:], func=mybir.ActivationFunctionType.Exp,
                scale=-1.0,
            ).then_inc(asem, 1)
            for c in range(CSH):
                s = c % NS
                scalar.wait_ge(fsem, c + 1)
                scalar.dma_start(out=out_re[c], in_=slot_ap(c)).then_inc(
                    ssem[s], 16
                )
            # drain: the kernel must not end with store DMAs in flight
            for s in range(NS):
                n_stores = len(range(s, CSH, NS))
                scalar.wait_ge(ssem[s], 16 * n_stores)

        @block.vector
        def _(vector):
            vector.wait_ge(ein, 64)  # et1, et2, cw1, cw2 (full count: exact)
            # band sums: e_b is laid out j-major ([C, S, BW] on host), so the
            # 21-tap sum is ONE innermost-axis tensor_reduce per half
            # (mean's 1/21 folded into cw on host)
            for (band, et, p) in ((band1, et1, 128), (band2, et2, 64)):
                etv = et.ap()[0:p, :].rearrange("p (j k) -> p j k", k=BW)
                vector.tensor_reduce(
                    out=band[0:p, 3 : 3 + S], in_=etv,
                    axis=mybir.AxisListType.X, op=Alu.add,
                )
            vector.memset(band1[:, 0:3], 0.0)
            vector.memset(band1[:, 3 + S :], 0.0)
            vector.memset(band2[:, 0:3], 0.0)
            vector.memset(band2[:, 3 + S :], 0.0)
            vector.memset(ct2[64:128, :], 0.0).then_inc(vs, 1)  # et region free
            # depthwise conv, 7 taps
            for (ct, band, cwt, p) in ((ct1, band1, cw1, 128), (ct2, band2, cw2, 64)):
                vector.tensor_scalar(
                    out=ct[0:p, :], in0=band[0:p, 0:S],
                    scalar1=cwt[0:p, 0:1], scalar2=None, op0=Alu.mult,
                )
                for t in range(1, K):
                    stt = vector.scalar_tensor_tensor(
                        out=ct[0:p, :], in0=band[0:p, t : t + S],
                        scalar=cwt[0:p, t : t + 1], in1=ct[0:p, :],
                        op0=Alu.mult, op1=Alu.add,
                    )
                stt.then_inc(vs, 1)  # vs=2 after ct1, vs=3 after ct2
            vector.wait_ge(psem, 1)
            vector.wait_ge(din, 80)  # pbt (and the rest; full count: exact)
            vector.tensor_scalar_add(out=sm[:], in0=ps[:], scalar1=pbt[:])
            vector.tensor_reduce(
                out=negmax[:], in_=sm[:], axis=mybir.AxisListType.X,
                op=Alu.max, negate=True,
            ).then_inc(vs, 1)  # vs=4: exp inputs ready
            vector.wait_ge(asem, 1)
            vector.tensor_reduce(
                out=ssum[:], in_=ex[:], axis=mybir.AxisListType.X, op=Alu.add
            ).then_inc(vs, 1)  # vs=5: ssum ready for ACT's 1/x seed
            vector.wait_ge(asem, 2)
            for _ in range(2):  # Newton: y <- y*(2 - x*y)
                vector.tensor_tensor(
                    out=nrt[:], in0=ssum[:], in1=rinv[:], op=Alu.mult
                )
                vector.tensor_scalar(
                    out=nrt[:], in0=nrt[:], scalar1=-1.0, scalar2=2.0,
                    op0=Alu.mult, op1=Alu.add,
                )
                vector.tensor_tensor(
                    out=rinv[:], in0=rinv[:], in1=nrt[:], op=Alu.mult
                )
            vector.tensor_scalar_mul(
                out=attn[:], in0=ex[:], scalar1=rinv[:]
            ).then_inc(vs, 1)  # vs=6: attn ready for PE transposes
            vector.wait_ge(psem, 2)
            vector.tensor_scalar_add(out=qm1[:], in0=psq[:], scalar1=-1.0)
            m0v = AP(
                tensor=m0s.ap().tensor, offset=m0s.ap().offset,
                ap=[list(m0s.ap().ap[0]), [S, 4], [1, S]],
            )
            fb = fbuf.ap()
            fb4 = AP(
                tensor=fb.tensor, offset=fb.offset,
                ap=[list(fb.ap[0]), [S, 4], [1, S]],
            )
            for c in range(CSH):
                s = c % NS
                vector.wait_ge(lsem[s], 16 * (c // NS + 1))
                tile = slot_ap(c)
                t4 = AP(
                    tensor=tile.tensor, offset=tile.offset,
                    ap=[list(tile.ap[0]), [S, 4], [1, S]],
                )
                qsb = (
                    qm1.ap()[:, c : 4 * CSH : CSH]
                    .unsqueeze(2)
                    .to_broadcast([128, 4, S])
                )
                # F = qs*M0 (nonzero only at diag positions); tile = (F+1)*tile
                vector.tensor_tensor(out=fb4, in0=qsb, in1=m0v, op=Alu.mult)
                vector.scalar_tensor_tensor(
                    out=t4, in0=fb4, scalar=1.0, in1=t4,
                    op0=Alu.add, op1=Alu.mult,
                ).then_inc(fsem, 1)

        @block.tensor
        def _(tensor):
            tensor.wait_ge(din, 80)
            tensor.wait_ge(vs, 3)
            nc.tensor.matmul(ps[:], lhsT=pw1[:], rhs=ct1[:], start=True, stop=False)
            nc.tensor.matmul(
                ps[:], lhsT=pw2[:], rhs=ct2[:], start=False, stop=True
            ).then_inc(psem, 1)
            tensor.wait_ge(vs, 6)
            # qm1[p, r*48+c] <- attn[c, 4p+r]: transpose the stride-4 slices
            for r in range(4):
                mm = nc.tensor.matmul(
                    psq[:, r * CSH : (r + 1) * CSH],
                    lhsT=attn[:, r : S : 4],
                    rhs=i48s[:],
                    start=True, stop=True,
                )
            mm.then_inc(psem, 1)  # psem=2: all transposes done

    return nc


def _get_program():
    if "p" not in _prog:
        _prog["p"] = _build_program()
    return _prog["p"]


def _host_prep(x, conv_w, point_w, point_b):
    """Build per-core input maps. Everything here is slicing/layout only."""
    x = np.asarray(x, dtype=np.float32)
    conv_w = np.asarray(conv_w, dtype=np.float32)
    point_w = np.asarray(point_w, dtype=np.float32)
    point_b = np.asarray(point_b, dtype=np.float32)

    # E[b,c,k,j] = xpad[b,c,j+k,j]  (rows padded by HALF), via diagonal views;
    # shipped j-major ([C, S, BW]) so the device band sum is one X-axis reduce
    E = np.zeros((B, C, BW, S), dtype=np.float32)
    for k in range(BW):
        o = HALF - k
        d = np.diagonal(x, offset=o, axis1=2, axis2=3)
        if o >= 0:
            E[:, :, k, o:S] = d
        else:
            E[:, :, k, 0 : S + o] = d
    E = np.ascontiguousarray(E.transpose(0, 1, 3, 2)).reshape(B, C, EBF)

    cw_all = np.ascontiguousarray(conv_w.reshape(C, K) / np.float32(BW))
    eye48 = np.eye(CSH, dtype=np.float32)
    # mask of diagonal positions in the [128, 4*S] tile layout:
    # partition p holds map rows 4p+r; row 4p+r's diagonal is at column 4p+r
    m0 = np.zeros((128, 4 * S), dtype=np.float32)
    for p in range(128):
        for r in range(4):
            m0[p, r * S + 4 * p + r] = 1.0

    in_maps = []
    for core in range(N_CORES):
        b, cb = divmod(core, 4)
        c0 = cb * CSH
        pwt_sh = np.zeros((256, CSH), dtype=np.float32)
        pwt_sh[:C] = point_w[c0 : c0 + CSH, :].T
        in_maps.append(
            {
                "x_sh": np.ascontiguousarray(x[b, c0 : c0 + CSH]),
                "e_b": np.ascontiguousarray(E[b]),
                "cw": cw_all,
                "pwt": pwt_sh,
                "pb": np.ascontiguousarray(point_b[c0 : c0 + CSH].reshape(CSH, 1)),
                "i48": eye48,
                "m0": m0,
            }
        )
    return in_maps


def _run(inputs, trace=False):
    from concourse.bass_utils import run_bass_kernel_spmd

    nc = _get_program()
    in_maps = _host_prep(**inputs)
    res = run_bass_kernel_spmd(
        nc, in_maps, core_ids=list(range(N_CORES)), trace=trace
    )
    out = np.empty((B, C, S, S), dtype=np.float32)
    for core in range(N_CORES):
        b, cb = divmod(core, 4)
        c0 = cb * CSH
        out[b, c0 : c0 + CSH] = res.results[core]["out"]
    return out, res


def kernel(x, conv_w, point_w, point_b):
    out, _ = _run(dict(x=x, conv_w=conv_w, point_w=point_w, point_b=point_b))
    return out
